# revision 21
# baseline (speedup 1.0000x reference)
"""Trainium2 Bass kernel: 16-head causal attention with RoPE (seq=4096, feat=1024).

Sharding: tensor-parallel on heads — 2 heads per core across 8 NeuronCores.
Each core computes the qkv projection for its 2 heads, RoPE, causal softmax
attention, and writes a (2*65, 4096) output slab (head outputs transposed,
plus fused softmax denominators); the host divides/transposes/concatenates.

Layout (TensorE contracts over the partition axis):
  - x arrives host-pre-transposed as xT (1024, 4096) bf16.
  - Stage 1: qkvT = W_shard.T @ xT in (c, s) layout; RoPE on the vector
    engine in f32 straight out of PSUM; V re-transposed to natural (s, d)
    layout via PE transpose (bf16) with an identity matrix.
  - Stage 2 (strictly after stage 1 — a cross-engine dep enforces the phase):
    scores transposed ST = K @ QT (k on partitions, q free), both heads
    concurrent via PE row tiling (K=64 at partition bases 0/64);
    P = exp(ST/8) on the scalar engine batched 3 score-tiles per op;
    causal masks multiplied on the diagonal tiles (bf16, DVE, SBUF-only);
    PV accumulated in PSUM as K=64 row-tile halves cross-paired across the
    two heads' banks so weight loads hide and streams pair up; the PV
    weights carry a ones column, accumulating softmax denominators for free.
  - While stage-2 row-split accumulation groups are in flight, every PSUM
    read goes through the scalar engine (a concurrent DVE PSUM read
    hard-crashes the exec unit — found empirically).
"""

import sys

if "/opt/trn_rl_repo" not in sys.path:
    sys.path.insert(0, "/opt/trn_rl_repo")

import numpy as np
import ml_dtypes

S = 4096
F = 1024
NH = 16
HD = 64
NCORES = 8
CH = 512          # q-chunk / psum bank free size in f32
NCHUNK = S // CH  # 8
KT = 128          # k-tile size
NKT = S // KT     # 32
VSLOT = 160       # vnat slot stride (elements); h0 V at +0, ones 64; h1 at +80
GRP = 3           # score tiles (kt, head) per exp op

_CACHE = {}


def _build_nc(debug_taps=False):
    import concourse.bass as bass
    import concourse.bacc as bacc
    import concourse.mybir as mybir
    import concourse.tile as tile

    f32 = mybir.dt.float32
    bf16 = mybir.dt.bfloat16
    EXP = mybir.ActivationFunctionType.Exp

    nc = bacc.Bacc("TRN2", target_bir_lowering=False, debug=False)

    xt_d = nc.dram_tensor("xt", [F, S], bf16, kind="ExternalInput")
    wq_d = nc.dram_tensor("wq", [F, 128], bf16, kind="ExternalInput")
    wk_d = nc.dram_tensor("wk", [F, 128], bf16, kind="ExternalInput")
    wv_d = nc.dram_tensor("wv", [F, 128], bf16, kind="ExternalInput")
    cos_d = nc.dram_tensor("cos", [128, S], f32, kind="ExternalInput")
    ss_d = nc.dram_tensor("ss", [128, S], f32, kind="ExternalInput")
    mask_d = nc.dram_tensor("mask", [128, 4 * CH], bf16, kind="ExternalInput")
    ident_d = nc.dram_tensor("ident", [128, 128], bf16, kind="ExternalInput")
    out_d = nc.dram_tensor("out", [130, S], f32, kind="ExternalOutput")
    if debug_taps:
        dbg_qt_d = nc.dram_tensor("dbg_qt", [128, S], bf16, kind="ExternalOutput")
        dbg_kt_d = nc.dram_tensor("dbg_kt", [128, S], bf16, kind="ExternalOutput")
        dbg_vn_d = nc.dram_tensor("dbg_vn", [128, NKT * VSLOT], bf16, kind="ExternalOutput")

    with tile.TileContext(nc) as tc:
        with (
            tc.tile_pool(name="const", bufs=1) as cpool,
            tc.tile_pool(name="persist", bufs=1) as perpool,
            tc.tile_pool(name="xt", bufs=16) as xpool,
            tc.tile_pool(name="rope", bufs=3) as rpool,
            tc.tile_pool(name="p", bufs=6) as ppool,
            tc.tile_pool(name="ob", bufs=4) as obpool,
        ):
            # ---- constants / persistent tiles ----
            cos_sb = cpool.tile([128, S], f32, tag="cos")
            ss_sb = cpool.tile([128, S], f32, tag="ss")
            mask_sb = cpool.tile([128, 4 * CH], bf16, tag="mask")
            wq_sb = cpool.tile([128, F], bf16, tag="wq")
            wk_sb = cpool.tile([128, F], bf16, tag="wk")
            wv_sb = cpool.tile([128, F], bf16, tag="wv")
            ident_sb = cpool.tile([128, 128], bf16, tag="ident")
            nc.sync.dma_start(cos_sb[:], cos_d[:])
            nc.sync.dma_start(ss_sb[:], ss_d[:])
            nc.sync.dma_start(mask_sb[:], mask_d[:])
            nc.sync.dma_start(ident_sb[:], ident_d[:])
            # W (1024, 128) -> lhsT tiles (128 f, 128 c) packed as (128, 8*128)
            for w_d, w_sb in ((wq_d, wq_sb), (wk_d, wk_sb), (wv_d, wv_sb)):
                nc.sync.dma_start(
                    w_sb[:].rearrange("p (t c) -> p t c", c=128),
                    w_d.rearrange("(t p) c -> p t c", p=128),
                )

            qT = perpool.tile([128, S], bf16, tag="qT")   # roped q, (d, s)
            kT = perpool.tile([128, S], bf16, tag="kT")   # roped k, (d, s)
            vnat = perpool.tile([128, NKT * VSLOT], bf16, tag="vnat")
            vnat3 = vnat.rearrange("p (t c) -> p t c", c=VSLOT)
            nc.vector.memset(vnat3[:, :, 64:65], 1.0)
            nc.vector.memset(vnat3[:, :, 144:145], 1.0)

            # ================= stage 1 =================
            last_s1 = None
            with tc.tile_pool(name="s1ps", bufs=2, space="PSUM") as s1pool:
                for c in range(NCHUNK):
                    sl = slice(c * CH, (c + 1) * CH)
                    xts = []
                    for ft in range(8):
                        t = xpool.tile([128, CH], bf16, tag="xt",
                                       name=f"xt{c}_{ft}")
                        nc.sync.dma_start(t[:], xt_d[ft * 128:(ft + 1) * 128, sl])
                        xts.append(t)
                    for ti, (w_sb, dest) in enumerate(
                        ((wq_sb, qT), (wk_sb, kT), (wv_sb, None))
                    ):
                        ps = s1pool.tile([128, CH], f32, tag="s1")
                        for ft in range(8):
                            nc.tensor.matmul(
                                ps[:],
                                lhsT=w_sb[:, ft * 128:(ft + 1) * 128],
                                rhs=xts[ft][:],
                                start=(ft == 0),
                                stop=(ft == 7),
                            )
                        if dest is not None:
                            # RoPE: rot = ps*cos + swap32(ps)*ss
                            sw = rpool.tile([128, CH], f32, tag="sw")
                            for b in range(4):
                                src = slice((b ^ 1) * 32, ((b ^ 1) + 1) * 32)
                                dst = slice(b * 32, (b + 1) * 32)
                                nc.vector.tensor_copy(sw[dst, :], ps[src, :])
                            t1 = rpool.tile([128, CH], f32, tag="t1")
                            t2 = rpool.tile([128, CH], f32, tag="t2")
                            nc.vector.tensor_mul(t1[:], ps[:], cos_sb[:, sl])
                            nc.vector.tensor_mul(t2[:], sw[:], ss_sb[:, sl])
                            last_s1 = nc.vector.tensor_add(
                                dest[:, sl], t1[:], t2[:]
                            )
                        else:
                            vbf = rpool.tile([128, CH], bf16, tag="vbf")
                            nc.vector.tensor_copy(vbf[:], ps[:])
                            for j in range(4):
                                kt = 4 * c + j
                                for h in range(2):
                                    tp = s1pool.tile(
                                        [128, 64], bf16, tag="s1",
                                        name=f"tp{kt}_{h}",
                                    )
                                    nc.tensor.transpose(
                                        tp[:],
                                        vbf[64 * h:64 * h + 64,
                                            j * 128:(j + 1) * 128],
                                        ident_sb[64 * h:64 * h + 64,
                                                 64 * h:64 * h + 64],
                                    )
                                    last_s1 = nc.vector.tensor_copy(
                                        vnat[:, kt * VSLOT + 80 * h:
                                             kt * VSLOT + 80 * h + 64],
                                        tp[:],
                                    )

            if debug_taps:
                nc.sync.dma_start(dbg_qt_d[:], qT[:])
                nc.sync.dma_start(dbg_kt_d[:], kT[:])
                nc.sync.dma_start(dbg_vn_d[:], vnat[:])

            # ================= stage 2 =================
            first_s2 = None
            with (
                tc.tile_pool(name="sps", bufs=2, space="PSUM") as spool,
                tc.tile_pool(name="ops", bufs=2, space="PSUM") as opool,
            ):
                for qc in range(NCHUNK):
                    qsl = slice(qc * CH, (qc + 1) * CH)
                    nkt = 4 * qc + 4
                    oT = [opool.tile([65, CH], f32, tag="oT",
                                     name=f"oT{qc}_{h}") for h in range(2)]
                    # flat stream of (kt, head) score tiles, GRP per exp op
                    tiles = [(kt, h) for kt in range(nkt) for h in range(2)]
                    pts = {}
                    for g0 in range(0, len(tiles), GRP):
                        grp = tiles[g0:g0 + GRP]
                        n = len(grp)
                        sps = spool.tile([128, GRP * CH], f32, tag="sps")
                        for j, (kt, h) in enumerate(grp):
                            mm = nc.tensor.matmul(
                                sps[:, j * CH:(j + 1) * CH],
                                lhsT=kT[64 * h:64 * h + 64,
                                        kt * KT:(kt + 1) * KT],
                                rhs=qT[64 * h:64 * h + 64, qsl],
                                start=True,
                                stop=True,
                            )
                            if first_s2 is None:
                                first_s2 = mm
                        pt = ppool.tile([128, GRP * CH], bf16, tag="pt",
                                        name=f"pt{qc}_{g0}")
                        nc.scalar.activation(
                            pt[:, :n * CH], sps[:, :n * CH], EXP,
                            scale=float(HD) ** -0.5,
                        )
                        for j, (kt, h) in enumerate(grp):
                            if kt >= 4 * qc:
                                m = kt - 4 * qc
                                nc.vector.tensor_mul(
                                    pt[:, j * CH:(j + 1) * CH],
                                    pt[:, j * CH:(j + 1) * CH],
                                    mask_sb[:, m * CH:(m + 1) * CH],
                                )
                            pts[(kt, h)] = (pt, j)
                        # PV for every kt whose both heads are ready
                        for kt in range(nkt):
                            if pts.get((kt, 0)) is not None \
                                    and pts.get((kt, 1)) is not None:
                                for h in range(2):
                                    spt, j = pts[(kt, h)]
                                    nc.tensor.matmul(
                                        oT[h][:],
                                        lhsT=vnat[:,
                                                  kt * VSLOT + 80 * h:
                                                  kt * VSLOT + 80 * h + 65],
                                        rhs=spt[:, j * CH:(j + 1) * CH],
                                        start=(kt == 0),
                                        stop=(kt == nkt - 1),
                                    )
                                pts[(kt, 0)] = None
                                pts[(kt, 1)] = None
                    for h in range(2):
                        ob = obpool.tile([65, CH], f32, tag="ob")
                        # scalar (ACT) copy: DVE must not read PSUM while
                        # row-split PV accumulation groups are in flight
                        nc.scalar.copy(ob[:], oT[h][:])
                        nc.sync.dma_start(out_d[65 * h:65 * h + 65, qsl], ob[:])

            # stage-2 PE work must not start until every stage-1 DVE
            # PSUM-read is done (same empirical hard-crash rule).
            if first_s2 is not None and last_s1 is not None:
                tile.add_dep_helper(
                    first_s2.ins, last_s1.ins,
                    reason="phase barrier: no DVE psum reads after "
                           "row-split PV groups start",
                )

    nc.compile()
    return nc


def _host_inputs(x, W_kqv, b_kqv):
    """Per-core input maps. Host work is layout/constants only."""
    f32 = np.float32
    bf16 = ml_dtypes.bfloat16
    xT = np.ascontiguousarray(x.T).astype(bf16)

    ts = (10000.0 ** (2.0 * np.arange(32) / HD)).astype(np.float64)
    pos = np.arange(S, dtype=np.float64)
    ang = pos[None, :] / ts[:, None]            # (32, S)
    cos32 = np.cos(ang)
    sin32 = np.sin(ang)
    cos128 = np.tile(cos32, (4, 1)).astype(f32)
    sgn = np.where((np.arange(128) % 64) < 32, -1.0, 1.0)[:, None]
    ss128 = (np.tile(sin32, (4, 1)) * sgn).astype(f32)

    ident = np.eye(128, dtype=bf16)
    ki = np.arange(128)[:, None]
    qi = np.arange(CH)[None, :]
    mask = np.concatenate(
        [(ki + 128 * j <= qi).astype(f32) for j in range(4)], axis=1
    ).astype(bf16)  # (128, 2048)

    in_maps = []
    for i in range(NCORES):
        in_maps.append({
            "xt": xT,
            "wq": np.ascontiguousarray(W_kqv[:, 128 * i:128 * i + 128]).astype(bf16),
            "wk": np.ascontiguousarray(W_kqv[:, F + 128 * i:F + 128 * i + 128]).astype(bf16),
            "wv": np.ascontiguousarray(W_kqv[:, 2 * F + 128 * i:2 * F + 128 * i + 128]).astype(bf16),
            "cos": cos128,
            "ss": ss128,
            "mask": mask,
            "ident": ident,
        })
    return in_maps


def _assemble(results):
    y = np.empty((S, F), np.float32)
    for i in range(NCORES):
        o = results[i]["out"]  # (130, S)
        for h in range(2):
            num = o[65 * h:65 * h + 64, :]
            den = o[65 * h + 64:65 * h + 65, :]
            hg = 2 * i + h
            y[:, HD * hg:HD * hg + HD] = (num / den).T
    return y


def kernel(x, W_kqv, b_kqv):
    from concourse import bass_utils

    if "nc" not in _CACHE:
        _CACHE["nc"] = _build_nc()
    nc = _CACHE["nc"]
    in_maps = _host_inputs(np.asarray(x), np.asarray(W_kqv), np.asarray(b_kqv))
    res = bass_utils.run_bass_kernel_spmd(nc, in_maps, core_ids=list(range(NCORES)))
    return _assemble(res.results)


# revision 22
# speedup vs baseline: 1.1405x; 1.1405x over previous
"""Trainium2 Bass kernel: 16-head causal attention with RoPE (seq=4096, feat=1024).

Sharding: tensor-parallel on heads — 2 heads per core across 8 NeuronCores.
Each core computes the qkv projection for its 2 heads, RoPE, causal softmax
attention, and writes a (2*65, 4096) output slab (head outputs transposed,
plus fused softmax denominators); the host divides/transposes/concatenates.

Layout (TensorE contracts over the partition axis):
  - x arrives host-pre-transposed as xT (1024, 4096) bf16.
  - Stage 1: qkvT = W_shard.T @ xT in (c, s) layout; RoPE on the vector
    engine in f32 straight out of PSUM; V re-transposed to natural (s, d)
    layout via PE transpose (bf16) with an identity matrix.
  - Stage 2 (strictly after stage 1 — a cross-engine dep enforces the phase):
    scores transposed ST = K @ QT (k on partitions, q free), both heads
    concurrent via PE row tiling (K=64 at partition bases 0/64);
    P = exp(ST/8) on the scalar engine batched 3 score-tiles per op;
    causal masks multiplied on the diagonal tiles (bf16, DVE, SBUF-only);
    PV accumulated in PSUM as K=64 row-tile halves cross-paired across the
    two heads' banks so weight loads hide and streams pair up; the PV
    weights carry a ones column, accumulating softmax denominators for free.
  - While stage-2 row-split accumulation groups are in flight, every PSUM
    read goes through the scalar engine (a concurrent DVE PSUM read
    hard-crashes the exec unit — found empirically).
"""

import sys

if "/opt/trn_rl_repo" not in sys.path:
    sys.path.insert(0, "/opt/trn_rl_repo")

import numpy as np
import ml_dtypes

S = 4096
F = 1024
NH = 16
HD = 64
NCORES = 8
CH = 512          # q-chunk / psum bank free size in f32
NCHUNK = S // CH  # 8
KT = 128          # k-tile size
NKT = S // KT     # 32
VSLOT = 160       # vnat slot stride (elements); h0 V at +0, ones 64; h1 at +80
GRP = 2           # score tiles (kt, head) per exp op

_CACHE = {}


def _build_nc(debug_taps=False):
    import concourse.bass as bass
    import concourse.bacc as bacc
    import concourse.mybir as mybir
    import concourse.tile as tile

    f32 = mybir.dt.float32
    bf16 = mybir.dt.bfloat16
    EXP = mybir.ActivationFunctionType.Exp

    nc = bacc.Bacc("TRN2", target_bir_lowering=False, debug=False)

    xt_d = nc.dram_tensor("xt", [F, S], bf16, kind="ExternalInput")
    wq_d = nc.dram_tensor("wq", [F, 128], bf16, kind="ExternalInput")
    wk_d = nc.dram_tensor("wk", [F, 128], bf16, kind="ExternalInput")
    wv_d = nc.dram_tensor("wv", [F, 128], bf16, kind="ExternalInput")
    cos_d = nc.dram_tensor("cos", [128, S], f32, kind="ExternalInput")
    ss_d = nc.dram_tensor("ss", [128, S], f32, kind="ExternalInput")
    mask_d = nc.dram_tensor("mask", [128, 4 * CH], bf16, kind="ExternalInput")
    ident_d = nc.dram_tensor("ident", [128, 128], bf16, kind="ExternalInput")
    out_d = nc.dram_tensor("out", [130, S], f32, kind="ExternalOutput")
    if debug_taps:
        dbg_qt_d = nc.dram_tensor("dbg_qt", [128, S], bf16, kind="ExternalOutput")
        dbg_kt_d = nc.dram_tensor("dbg_kt", [128, S], bf16, kind="ExternalOutput")
        dbg_vn_d = nc.dram_tensor("dbg_vn", [128, NKT * VSLOT], bf16, kind="ExternalOutput")

    with tile.TileContext(nc) as tc:
        with (
            tc.tile_pool(name="const", bufs=1) as cpool,
            tc.tile_pool(name="persist", bufs=1) as perpool,
            tc.tile_pool(name="xt", bufs=16) as xpool,
            tc.tile_pool(name="rope", bufs=3) as rpool,
            tc.tile_pool(name="p", bufs=6) as ppool,
            tc.tile_pool(name="ob", bufs=4) as obpool,
            tc.tile_pool(name="s1ps", bufs=1, space="PSUM") as s1pool,
            tc.tile_pool(name="sps", bufs=2, space="PSUM") as spool,
            tc.tile_pool(name="ops", bufs=3, space="PSUM") as opool,
        ):
            # ---- constants / persistent tiles ----
            cos_sb = cpool.tile([128, S], f32, tag="cos")
            ss_sb = cpool.tile([128, S], f32, tag="ss")
            mask_sb = cpool.tile([128, 4 * CH], bf16, tag="mask")
            wq_sb = cpool.tile([128, F], bf16, tag="wq")
            wk_sb = cpool.tile([128, F], bf16, tag="wk")
            wv_sb = cpool.tile([128, F], bf16, tag="wv")
            ident_sb = cpool.tile([128, 128], bf16, tag="ident")
            nc.sync.dma_start(cos_sb[:], cos_d[:])
            nc.sync.dma_start(ss_sb[:], ss_d[:])
            nc.sync.dma_start(mask_sb[:], mask_d[:])
            nc.sync.dma_start(ident_sb[:], ident_d[:])
            # W (1024, 128) -> lhsT tiles (128 f, 128 c) packed as (128, 8*128)
            for w_d, w_sb in ((wq_d, wq_sb), (wk_d, wk_sb), (wv_d, wv_sb)):
                nc.sync.dma_start(
                    w_sb[:].rearrange("p (t c) -> p t c", c=128),
                    w_d.rearrange("(t p) c -> p t c", p=128),
                )

            qT = perpool.tile([128, S], bf16, tag="qT")   # roped q, (d, s)
            kT = perpool.tile([128, S], bf16, tag="kT")   # roped k, (d, s)
            vnat = perpool.tile([128, NKT * VSLOT], bf16, tag="vnat")
            vnat3 = vnat.rearrange("p (t c) -> p t c", c=VSLOT)
            nc.vector.memset(vnat3[:, :, 64:65], 1.0)
            nc.vector.memset(vnat3[:, :, 144:145], 1.0)

            # ================= stage 1 =================
            last_s1 = None
            if True:
                for c in range(NCHUNK):
                    sl = slice(c * CH, (c + 1) * CH)
                    xts = []
                    for ft in range(8):
                        t = xpool.tile([128, CH], bf16, tag="xt",
                                       name=f"xt{c}_{ft}")
                        nc.sync.dma_start(t[:], xt_d[ft * 128:(ft + 1) * 128, sl])
                        xts.append(t)
                    for ti, (w_sb, dest) in enumerate(
                        ((wq_sb, qT), (wk_sb, kT), (wv_sb, None))
                    ):
                        ps = s1pool.tile([128, CH], f32, tag="s1")
                        for ft in range(8):
                            nc.tensor.matmul(
                                ps[:],
                                lhsT=w_sb[:, ft * 128:(ft + 1) * 128],
                                rhs=xts[ft][:],
                                start=(ft == 0),
                                stop=(ft == 7),
                            )
                        if dest is not None:
                            # RoPE: rot = ps*cos + swap32(ps)*ss
                            sw = rpool.tile([128, CH], f32, tag="sw")
                            for b in range(4):
                                src = slice((b ^ 1) * 32, ((b ^ 1) + 1) * 32)
                                dst = slice(b * 32, (b + 1) * 32)
                                nc.vector.tensor_copy(sw[dst, :], ps[src, :])
                            t1 = rpool.tile([128, CH], f32, tag="t1")
                            t2 = rpool.tile([128, CH], f32, tag="t2")
                            nc.vector.tensor_mul(t1[:], ps[:], cos_sb[:, sl])
                            nc.vector.tensor_mul(t2[:], sw[:], ss_sb[:, sl])
                            last_s1 = nc.vector.tensor_add(
                                dest[:, sl], t1[:], t2[:]
                            )
                        else:
                            vbf = rpool.tile([128, CH], bf16, tag="vbf")
                            nc.vector.tensor_copy(vbf[:], ps[:])
                            for j in range(4):
                                kt = 4 * c + j
                                for h in range(2):
                                    tp = s1pool.tile(
                                        [128, 64], bf16, tag="s1",
                                        name=f"tp{kt}_{h}",
                                    )
                                    nc.tensor.transpose(
                                        tp[:],
                                        vbf[64 * h:64 * h + 64,
                                            j * 128:(j + 1) * 128],
                                        ident_sb[64 * h:64 * h + 64,
                                                 64 * h:64 * h + 64],
                                    )
                                    last_s1 = nc.vector.tensor_copy(
                                        vnat[:, kt * VSLOT + 80 * h:
                                             kt * VSLOT + 80 * h + 64],
                                        tp[:],
                                    )

            if debug_taps:
                nc.sync.dma_start(dbg_qt_d[:], qT[:])
                nc.sync.dma_start(dbg_kt_d[:], kT[:])
                nc.sync.dma_start(dbg_vn_d[:], vnat[:])

            # ================= stage 2 =================
            first_s2 = None
            if True:
                for qc in range(NCHUNK):
                    qsl = slice(qc * CH, (qc + 1) * CH)
                    nkt = 4 * qc + 4
                    oT = [opool.tile([65, CH], f32, tag="oT",
                                     name=f"oT{qc}_{h}") for h in range(2)]
                    # flat stream of (kt, head) score tiles, GRP per exp op
                    tiles = [(kt, h) for kt in range(nkt) for h in range(2)]
                    pts = {}
                    for g0 in range(0, len(tiles), GRP):
                        grp = tiles[g0:g0 + GRP]
                        n = len(grp)
                        sps = spool.tile([128, GRP * CH], f32, tag="sps")
                        for j, (kt, h) in enumerate(grp):
                            mm = nc.tensor.matmul(
                                sps[:, j * CH:(j + 1) * CH],
                                lhsT=kT[64 * h:64 * h + 64,
                                        kt * KT:(kt + 1) * KT],
                                rhs=qT[64 * h:64 * h + 64, qsl],
                                start=True,
                                stop=True,
                            )
                            if first_s2 is None:
                                first_s2 = mm
                        pt = ppool.tile([128, GRP * CH], bf16, tag="pt",
                                        name=f"pt{qc}_{g0}")
                        nc.scalar.activation(
                            pt[:, :n * CH], sps[:, :n * CH], EXP,
                            scale=float(HD) ** -0.5,
                        )
                        for j, (kt, h) in enumerate(grp):
                            if kt >= 4 * qc:
                                m = kt - 4 * qc
                                nc.vector.tensor_mul(
                                    pt[:, j * CH:(j + 1) * CH],
                                    pt[:, j * CH:(j + 1) * CH],
                                    mask_sb[:, m * CH:(m + 1) * CH],
                                )
                            pts[(kt, h)] = (pt, j)
                        # PV for every kt whose both heads are ready
                        for kt in range(nkt):
                            if pts.get((kt, 0)) is not None \
                                    and pts.get((kt, 1)) is not None:
                                # diagonal tiles only contribute masked
                                # zeros below q = 128*m: narrow the stream
                                m = max(kt - 4 * qc, 0)
                                for h in range(2):
                                    spt, j = pts[(kt, h)]
                                    nc.tensor.matmul(
                                        oT[h][0:65, 128 * m:CH],
                                        lhsT=vnat[:,
                                                  kt * VSLOT + 80 * h:
                                                  kt * VSLOT + 80 * h + 65],
                                        rhs=spt[:, j * CH + 128 * m:
                                                (j + 1) * CH],
                                        start=(kt == 0),
                                        stop=(kt == nkt - 1),
                                    )
                                pts[(kt, 0)] = None
                                pts[(kt, 1)] = None
                    for h in range(2):
                        ob = obpool.tile([65, CH], f32, tag="ob")
                        nc.vector.tensor_copy(ob[:], oT[h][:])
                        nc.sync.dma_start(out_d[65 * h:65 * h + 65, qsl], ob[:])


    nc.compile()
    return nc


def _host_inputs(x, W_kqv, b_kqv):
    """Per-core input maps. Host work is layout/constants only."""
    f32 = np.float32
    bf16 = ml_dtypes.bfloat16
    xT = np.ascontiguousarray(x.T).astype(bf16)

    ts = (10000.0 ** (2.0 * np.arange(32) / HD)).astype(np.float64)
    pos = np.arange(S, dtype=np.float64)
    ang = pos[None, :] / ts[:, None]            # (32, S)
    cos32 = np.cos(ang)
    sin32 = np.sin(ang)
    cos128 = np.tile(cos32, (4, 1)).astype(f32)
    sgn = np.where((np.arange(128) % 64) < 32, -1.0, 1.0)[:, None]
    ss128 = (np.tile(sin32, (4, 1)) * sgn).astype(f32)

    ident = np.eye(128, dtype=bf16)
    ki = np.arange(128)[:, None]
    qi = np.arange(CH)[None, :]
    mask = np.concatenate(
        [(ki + 128 * j <= qi).astype(f32) for j in range(4)], axis=1
    ).astype(bf16)  # (128, 2048)

    in_maps = []
    for i in range(NCORES):
        in_maps.append({
            "xt": xT,
            "wq": np.ascontiguousarray(W_kqv[:, 128 * i:128 * i + 128]).astype(bf16),
            "wk": np.ascontiguousarray(W_kqv[:, F + 128 * i:F + 128 * i + 128]).astype(bf16),
            "wv": np.ascontiguousarray(W_kqv[:, 2 * F + 128 * i:2 * F + 128 * i + 128]).astype(bf16),
            "cos": cos128,
            "ss": ss128,
            "mask": mask,
            "ident": ident,
        })
    return in_maps


def _assemble(results):
    y = np.empty((S, F), np.float32)
    for i in range(NCORES):
        o = results[i]["out"]  # (130, S)
        for h in range(2):
            num = o[65 * h:65 * h + 64, :]
            den = o[65 * h + 64:65 * h + 65, :]
            hg = 2 * i + h
            y[:, HD * hg:HD * hg + HD] = (num / den).T
    return y


def kernel(x, W_kqv, b_kqv):
    from concourse import bass_utils

    if "nc" not in _CACHE:
        _CACHE["nc"] = _build_nc()
    nc = _CACHE["nc"]
    in_maps = _host_inputs(np.asarray(x), np.asarray(W_kqv), np.asarray(b_kqv))
    res = bass_utils.run_bass_kernel_spmd(nc, in_maps, core_ids=list(range(NCORES)))
    return _assemble(res.results)
